# revision 3
# baseline (speedup 1.0000x reference)
"""Fake-quantized (int8 STE) 3x3 SAME conv on 8 trn2 NeuronCores.

Strategy (data-parallel, hint-conformant):
  - shard lhs/weights along batch (4 images per core), replicate the 3x3x64x128
    filter, AllReduce(max) the per-tensor lhs calibration bound across cores.
  - after fake-quant both operands are integers in [-127,127]: exactly
    representable in bf16, and all conv partial sums < 2^24, so a bf16
    matmul accumulating in fp32 PSUM is bit-exact integer arithmetic.
  - conv is im2col with K=576 split into 128-deep partition chunks; the
    x-parity-split "TC" layout lets 4 of the chunks be single K=128 APs and
    pairs the two leftover K=64 chunks into disjoint PE row-groups that run
    concurrently -> ~4.5 streamed matmul slots per output position.
"""

import numpy as np
import os
ABL = os.environ.get('KABL', '')

import concourse.bass as bass
import concourse.tile as tile
from concourse import bacc, bass_isa, bass_utils, mybir

f32 = mybir.dt.float32
bf16 = mybir.dt.bfloat16

N_CORES = 8
N_PER_CORE = 4
H = W = 112
C = 64
CO = 128
ROWF = W * C            # 7168 floats per image row
HALF = ROWF // 2        # 3584 (quantize chunk)
NJ = 58                 # j' slots: j'=0 pad(x=-1 side), 1..56 data, 57 pad(x=112)
RSLOTS = H           # 112 row slots (no pad rows; boundary matmuls are clipped)
TCW = NJ * RSLOTS       # 6496, j-major: col = j'*112 + y
MAGIC = 12582912.0      # 1.5 * 2**23: fp32 add => round-to-nearest-even
CLIPV = 127.0
EPS = 1e-6
G_ROWS = 8              # output rows per psum tile
N_G = H // G_ROWS       # 14 row groups per image
SG = 3                  # row groups per supergroup (weight-reuse window)


def _weight_prep(nc, pool, wq_pool, rhs_ap):
    """Load the 12 filter chunk tiles (f32), return (wf tiles, per-row |W| max)."""
    wflat = rhs_ap.rearrange("a b i o -> (a b i) o")  # [576, 128], k-major
    wf = {}
    for ky in range(3):
        base = 192 * ky
        a = pool.tile([128, 128], f32, name=f"wfA{ky}")
        nc.sync.dma_start(a[:], wflat[base + 64: base + 192, :])
        b = pool.tile([128, 128], f32, name=f"wfB{ky}")
        nc.sync.dma_start(b[:], wflat[base: base + 128, :])
        u = pool.tile([128, 128], f32, name=f"wfU{ky}")
        nc.sync.dma_start(u[64:128, :], wflat[base: base + 64, :])
        low = pool.tile([128, 128], f32, name=f"wfL{ky}")
        nc.sync.dma_start(low[0:64, :], wflat[base + 128: base + 192, :])
        wf[("A", ky)] = a
        wf[("B", ky)] = b
        wf[("U", ky)] = u
        wf[("L", ky)] = low

    # |W| max: B0..B2 cover taps (ky,0),(ky,1); L0..L2 cover (ky,2) on parts 0:64
    mb = []
    for ky in range(3):
        m = pool.tile([128, 1], f32, name=f"wmB{ky}")
        nc.vector.tensor_reduce(m[:], wf[("B", ky)][:], axis=mybir.AxisListType.X,
                                op=mybir.AluOpType.max, apply_absolute_value=True)
        mb.append(m)
        ml = pool.tile([128, 1], f32, name=f"wmL{ky}")
        nc.vector.tensor_reduce(ml[0:64, :], wf[("L", ky)][0:64, :],
                                axis=mybir.AxisListType.X,
                                op=mybir.AluOpType.max, apply_absolute_value=True)
        mb.append(None)
        wf[("Lmax", ky)] = ml
    wmax = pool.tile([128, 1], f32, name="wmax")
    nc.vector.tensor_tensor(wmax[:], mb[0][:], mb[2][:], op=mybir.AluOpType.max)
    nc.vector.tensor_tensor(wmax[:], wmax[:], mb[4][:], op=mybir.AluOpType.max)
    for ky in range(3):
        nc.vector.tensor_tensor(wmax[0:64, :], wmax[0:64, :],
                                wf[("Lmax", ky)][0:64, :], op=mybir.AluOpType.max)
    wbound = pool.tile([128, 1], f32, name="wbound")
    nc.gpsimd.partition_all_reduce(wbound[:], wmax[:], channels=128,
                                   reduce_op=bass_isa.ReduceOp.max)
    return wf, wbound


def _quantize_tile(nc, tmp_pool, dst, src, scale_ap):
    """dst(bf16) = round_half_even(src * scale), via the fp32 magic constant."""
    p = src.shape[0]
    tmpq = tmp_pool.tile(list(src.shape), f32, name="tmpq")
    nc.vector.tensor_scalar(tmpq[:], src, scale_ap[0:p, :], MAGIC,
                            op0=mybir.AluOpType.mult, op1=mybir.AluOpType.add)
    nc.vector.tensor_scalar(dst, tmpq[:], MAGIC, None,
                            op0=mybir.AluOpType.subtract)


def build_bass(n_cores=N_CORES, n_img=N_PER_CORE, repeat=1):
    nc = bacc.Bacc("TRN2", target_bir_lowering=False, debug=False,
                   num_devices=n_cores)
    lhs = nc.dram_tensor("lhs", [n_img, H, W, C], f32, kind="ExternalInput").ap()
    rhs = nc.dram_tensor("rhs", [3, 3, C, CO], f32, kind="ExternalInput").ap()
    wts = nc.dram_tensor("weights", [n_img, 1, 1, 1], f32, kind="ExternalInput").ap()
    out = nc.dram_tensor("out", [n_img, H, W, CO], f32, kind="ExternalOutput").ap()

    with tile.TileContext(nc) as tc:
        for _ in range(repeat):
            _body(tc, nc, lhs, rhs, wts, out, n_cores, n_img)
    nc.compile()
    return nc


def _body(tc, nc, lhs, rhs, wts, out, n_cores, n_img):
    from contextlib import ExitStack
    with ExitStack() as ctx:
        cpool = ctx.enter_context(tc.tile_pool(name="cpool", bufs=1))
        wq_pool = ctx.enter_context(tc.tile_pool(name="wq", bufs=1))
        trow_pool = ctx.enter_context(tc.tile_pool(name="trow", bufs=n_img))
        tmp_pool = ctx.enter_context(tc.tile_pool(name="tmpq", bufs=2))
        tq_pool = ctx.enter_context(tc.tile_pool(name="tq", bufs=2))
        tc_pool = ctx.enter_context(tc.tile_pool(name="tcq", bufs=2))
        st_pool = ctx.enter_context(tc.tile_pool(name="stg", bufs=3))
        st2_pool = ctx.enter_context(tc.tile_pool(name="stg2", bufs=3))
        ps_pool = ctx.enter_context(tc.tile_pool(name="ps", bufs=6, space="PSUM"))
        ps2_pool = ctx.enter_context(tc.tile_pool(name="ps2", bufs=2, space="PSUM"))
        dram_pool = ctx.enter_context(tc.tile_pool(name="dram", bufs=1, space="DRAM"))

        # ---- constants
        c127 = cpool.tile([128, 1], f32, name="c127")
        nc.vector.memset(c127[:], CLIPV)
        c1 = cpool.tile([128, 1], f32, name="c1")
        nc.vector.memset(c1[:], 1.0)

        # ---- filter prep (local; every core computes the same scale)
        wf, wbound = _weight_prep(nc, cpool, wq_pool, rhs)

        # ---- load lhs (canonical: partition=row) + masked abs-max
        trows = []
        mcat = cpool.tile([1, max(n_img, 2)], f32, name="mcat")
        lhs_rows = lhs.rearrange("n y x c -> n y (x c)")  # [n, 112, 7168]
        for i in range(n_img):
            t = trow_pool.tile([H, ROWF], f32, name="trow")
            nc.sync.dma_start(t[:], lhs_rows[i])
            trows.append(t)
            m = cpool.tile([H, 1], f32, name=f"lm{i}")
            nc.vector.tensor_reduce(m[:], t[:], axis=mybir.AxisListType.X,
                                    op=mybir.AluOpType.max,
                                    apply_absolute_value=True)
            pm = cpool.tile([H, 1], f32, name=f"lpm{i}")
            nc.gpsimd.partition_all_reduce(pm[:], m[:], channels=H,
                                           reduce_op=bass_isa.ReduceOp.max)
            nc.vector.tensor_copy(mcat[0:1, i:i + 1], pm[0:1, :])

        # mask = weights > 0 (per image)
        wcal = cpool.tile([1, max(n_img, 2)], f32, name="wcal")
        nc.sync.dma_start(wcal[0:1, 0:n_img],
                          wts.rearrange("n a b c -> (a b c) n"))
        ind = cpool.tile([1, max(n_img, 2)], f32, name="ind")
        nc.vector.tensor_scalar(ind[0:1, 0:n_img], wcal[0:1, 0:n_img], 0.0, None,
                                op0=mybir.AluOpType.is_gt)
        masked = cpool.tile([1, max(n_img, 2)], f32, name="masked")
        nc.vector.tensor_tensor(masked[0:1, 0:n_img], mcat[0:1, 0:n_img],
                                ind[0:1, 0:n_img], op=mybir.AluOpType.mult)
        lb = cpool.tile([1, 1], f32, name="lb")
        nc.vector.tensor_reduce(lb[:], masked[0:1, 0:n_img],
                                axis=mybir.AxisListType.X, op=mybir.AluOpType.max)

        # ---- global max across cores
        if n_cores > 1 and 'nocc' not in ABL:
            cc_in = dram_pool.tile([1, 1], f32, name="cc_in")
            cc_space = "Shared" if n_cores > 4 else "Local"
            cc_out = dram_pool.tile([1, 1], f32, name="cc_out",
                                    addr_space=cc_space)
            nc.gpsimd.dma_start(cc_in[:], lb[:])
            nc.gpsimd.collective_compute(
                "AllReduce", mybir.AluOpType.max,
                replica_groups=[list(range(n_cores))],
                ins=[cc_in.opt()], outs=[cc_out.opt()])
            gb0 = cpool.tile([1, 1], f32, name="gb0")
            nc.gpsimd.dma_start(gb0[:], cc_out[:])
        else:
            gb0 = lb

        gbb = cpool.tile([128, 1], f32, name="gbb")
        nc.gpsimd.partition_broadcast(gbb[:], gb0[:])
        gb2 = cpool.tile([128, 1], f32, name="gb2")
        nc.vector.tensor_scalar(gb2[:], gbb[:], EPS, None,
                                op0=mybir.AluOpType.max)
        rls = cpool.tile([128, 1], f32, name="rls")
        nc.vector.reciprocal(rls[:], gb2[:])
        ls = cpool.tile([128, 1], f32, name="ls")  # lhs_scale = 127/bound
        nc.vector.tensor_scalar(ls[:], rls[:], CLIPV, None,
                                op0=mybir.AluOpType.mult)

        wb2 = cpool.tile([128, 1], f32, name="wb2")
        nc.vector.tensor_scalar(wb2[:], wbound[:], EPS, None,
                                op0=mybir.AluOpType.max)
        rws = cpool.tile([128, 1], f32, name="rws")
        nc.vector.reciprocal(rws[:], wb2[:])
        ws = cpool.tile([128, 1], f32, name="ws")   # rhs_scale
        nc.vector.tensor_scalar(ws[:], rws[:], CLIPV, None,
                                op0=mybir.AluOpType.mult)
        sprod = cpool.tile([128, 1], f32, name="sprod")
        nc.vector.tensor_tensor(sprod[:], ls[:], ws[:], op=mybir.AluOpType.mult)
        dq = cpool.tile([128, 1], f32, name="dq")   # 1/(ls*ws)
        nc.vector.reciprocal(dq[:], sprod[:])

        # ---- quantize filter chunks -> bf16 lhsT tiles
        wq = {}
        for key_kind in ("A", "B", "U", "L"):
            for ky in range(3):
                src = wf[(key_kind, ky)]
                dst = wq_pool.tile([128, 128], bf16, name=f"wq{key_kind}{ky}")
                if key_kind == "U":
                    _quantize_tile(nc, tmp_pool, dst[64:128, :],
                                   src[64:128, :], ws[64:128, :])
                elif key_kind == "L":
                    _quantize_tile(nc, tmp_pool, dst[0:64, :],
                                   src[0:64, :], ws)
                else:
                    _quantize_tile(nc, tmp_pool, dst[:], src[:], ws)
                wq[(key_kind, ky)] = dst

        wzero = wq_pool.tile([128, 128], bf16, name="wzero")
        nc.vector.memset(wzero[:], 0.0)

        # output view: x = 2*m + parity; dims (n, parity, c, y, m)
        from concourse.masks import make_identity
        ident = cpool.tile([128, 128], f32, name="ident")
        make_identity(nc, ident[:])

        # ---- per image: quantize lhs -> xbar-transpose into TC -> conv
        for i in range(n_img):
            tcq = tc_pool.tile([128, TCW], bf16, name="tcq")
            nc.gpsimd.memset(tcq[:], 0.0)
            # j-major views [p, j', y]; transpose dst contiguous per partition
            tcr = tcq.rearrange("p (j r) -> p j r", r=RSLOTS)  # [128,58,112]
            for h in range(2):
                tq = tq_pool.tile([H, HALF], bf16, name="tq")
                _quantize_tile(nc, tmp_pool, tq[:],
                               trows[i][:, h * HALF:(h + 1) * HALF], ls)
                nc.sync.dma_start_transpose(
                    tcr[:, 1 + 28 * h: 1 + 28 * (h + 1), :], tq[:])

            tcr2 = tcq.rearrange("p (j r) -> p r j", r=RSLOTS)  # [128,112,58]

            def clip(g, ky):
                lo_in = g * G_ROWS + ky - 1
                lo, hi = max(0, lo_in), min(H, lo_in + G_ROWS)
                return lo, hi, lo - lo_in, hi - lo_in

            KYS = (1, 0, 2)  # ky=1 first: full rows, starts the accumulation
            for sg0 in range(0, N_G, SG):
                gs = range(sg0, min(sg0 + SG, N_G))
                pse = {}
                pso = {}
                for g in gs:
                    pse[g] = ps_pool.tile([128, G_ROWS, 56], f32, name="pse",
                                          tag="psb")
                    pso[g] = ps_pool.tile([128, G_ROWS, 56], f32, name="pso",
                                          tag="psb")
                # full K=128 chunks (A: even-parity outputs, B: odd)
                for kind, psd in (() if 'nomm' in ABL else (("A", pse), ("B", pso))):
                    for ky in KYS:
                        for g in gs:
                            lo, hi, a, b = clip(g, ky)
                            nc.tensor.matmul(
                                psd[g][:, a:b, :], wq[(kind, ky)][:],
                                tcr2[:, lo:hi, 1:57],
                                start=(ky == 1), stop=False,
                                skip_group_check=True)
                # half chunks: U (upper rows, even) / L (lower rows, odd);
                # adjacent issue -> disjoint PE row-groups run concurrently
                for ky in (() if 'nomm' in ABL else KYS):
                    for g in gs:
                        lo, hi, a, b = clip(g, ky)
                        nc.tensor.matmul(
                            pse[g][:, a:b, :], wq[("U", ky)][64:128, :],
                            tcr2[64:128, lo:hi, 0:56],
                            start=False, stop=(ky == 2),
                            skip_group_check=True)
                        nc.tensor.matmul(
                            pso[g][:, a:b, :], wq[("L", ky)][0:64, :],
                            tcr2[0:64, lo:hi, 2:58],
                            start=False, stop=(ky == 2),
                            skip_group_check=True)
                # dequant, TensorE-transpose to [spatial, ch], store
                for g in (() if 'noout' in ABL else gs):
                    for parity, ps in ((0, pse[g]), (1, pso[g])):
                        st = st_pool.tile([128, 448], f32, name="stg")
                        nc.scalar.activation(st[:], ps.rearrange("p a b -> p (a b)"),
                                             mybir.ActivationFunctionType.Copy,
                                             scale=dq[:])
                        st2 = st2_pool.tile([112, 4, 128], f32, name="stg2")
                        ps2 = ps2_pool.tile([112, 512], f32, name="ps2")
                        for t in range(4):
                            nc.tensor.transpose(ps2[:, 128 * t:128 * (t + 1)],
                                                st[:, 112 * t:112 * (t + 1)],
                                                ident[:])
                        st2f = st2.rearrange("p a b -> p (a b)")
                        if parity == 0:
                            nc.vector.tensor_copy(st2f, ps2[:])
                        else:
                            nc.scalar.copy(st2f, ps2[:])
                        # out[i, y, x, c]: x=(t*14+dm)*2+parity; p=(dm,y)
                        dview = out[i, g * G_ROWS:(g + 1) * G_ROWS] \
                            .rearrange("(t dy) (m two) c -> two dy m t c",
                                       t=4, two=2)[parity]
                        nc.sync.dma_start(dview, st2[:])


_NC_CACHE = {}


def _get_nc(repeat=1):
    key = (N_CORES, N_PER_CORE, repeat)
    if key not in _NC_CACHE:
        _NC_CACHE[key] = build_bass(N_CORES, N_PER_CORE, repeat=repeat)
    return _NC_CACHE[key]


def kernel(lhs, rhs, weights, event_count=None, event_count_for_filter=None,
           **_ignored):
    lhs = np.ascontiguousarray(np.asarray(lhs, dtype=np.float32))
    rhs = np.ascontiguousarray(np.asarray(rhs, dtype=np.float32))
    weights = np.ascontiguousarray(np.asarray(weights, dtype=np.float32))
    n = lhs.shape[0]
    per = n // N_CORES
    assert per == N_PER_CORE and n % N_CORES == 0

    nc = _get_nc()
    in_maps = []
    for c in range(N_CORES):
        in_maps.append({
            "lhs": lhs[c * per:(c + 1) * per],
            "rhs": rhs,
            "weights": weights[c * per:(c + 1) * per],
        })
    res = bass_utils.run_bass_kernel_spmd(nc, in_maps,
                                          core_ids=list(range(N_CORES)))
    return np.concatenate([res.results[c]["out"] for c in range(N_CORES)],
                          axis=0)



# revision 13
# speedup vs baseline: 1.6586x; 1.6586x over previous
"""Fake-quantized (int8 STE) 3x3 SAME conv on 8 trn2 NeuronCores.

Strategy (data-parallel, hint-conformant):
  - shard lhs/weights along batch (4 images per core), replicate the 3x3x64x128
    filter, AllReduce(max) the per-tensor lhs calibration bound across cores.
  - after fake-quant both operands are integers in [-127,127]: exactly
    representable in bf16, and all conv partial sums < 2^24, so a bf16
    matmul accumulating in fp32 PSUM is bit-exact integer arithmetic.
  - conv is im2col with K=576 split into 128-deep partition chunks; the
    x-parity-split "TC" layout lets 4 of the chunks be single K=128 APs and
    pairs the two leftover K=64 chunks into disjoint PE row-groups that run
    concurrently -> ~4.5 streamed matmul slots per output position.
"""

import numpy as np
import os
ABL = os.environ.get('KABL', '')

import concourse.bass as bass
import concourse.tile as tile
from concourse import bacc, bass_isa, bass_utils, mybir

f32 = mybir.dt.float32
bf16 = mybir.dt.bfloat16

N_CORES = 8
N_PER_CORE = 4
H = W = 112
C = 64
CO = 128
ROWF = W * C            # 7168 floats per image row
HALF = ROWF // 2        # 3584 (quantize chunk)
NJ = 58                 # j' slots: j'=0 pad(x=-1 side), 1..56 data, 57 pad(x=112)
RSLOTS = H           # 112 row slots (no pad rows; boundary matmuls are clipped)
TCW = NJ * RSLOTS       # 6496, j-major: col = j'*112 + y
MAGIC = 12582912.0      # 1.5 * 2**23: fp32 add => round-to-nearest-even
CLIPV = 127.0
EPS = 1e-6
G_ROWS = 8              # output rows per psum tile
N_G = H // G_ROWS       # 14 row groups per image
SG = 3                  # row groups per supergroup (weight-reuse window)


def _weight_prep(nc, pool, wq_pool, rhs_ap):
    """Load the 12 filter chunk tiles (f32), return (wf tiles, per-row |W| max)."""
    wflat = rhs_ap.rearrange("a b i o -> (a b i) o")  # [576, 128], k-major
    wf = {}
    for ky in range(3):
        base = 192 * ky
        a = pool.tile([128, 128], f32, name=f"wfA{ky}")
        nc.sync.dma_start(a[:], wflat[base + 64: base + 192, :])
        b = pool.tile([128, 128], f32, name=f"wfB{ky}")
        nc.sync.dma_start(b[:], wflat[base: base + 128, :])
        u = pool.tile([128, 128], f32, name=f"wfU{ky}")
        nc.sync.dma_start(u[64:128, :], wflat[base: base + 64, :])
        low = pool.tile([128, 128], f32, name=f"wfL{ky}")
        nc.sync.dma_start(low[0:64, :], wflat[base + 128: base + 192, :])
        wf[("A", ky)] = a
        wf[("B", ky)] = b
        wf[("U", ky)] = u
        wf[("L", ky)] = low

    # |W| max: B0..B2 cover taps (ky,0),(ky,1); L0..L2 cover (ky,2) on parts 0:64
    mb = []
    for ky in range(3):
        m = pool.tile([128, 1], f32, name=f"wmB{ky}")
        nc.vector.tensor_reduce(m[:], wf[("B", ky)][:], axis=mybir.AxisListType.X,
                                op=mybir.AluOpType.max, apply_absolute_value=True)
        mb.append(m)
        ml = pool.tile([128, 1], f32, name=f"wmL{ky}")
        nc.vector.tensor_reduce(ml[0:64, :], wf[("L", ky)][0:64, :],
                                axis=mybir.AxisListType.X,
                                op=mybir.AluOpType.max, apply_absolute_value=True)
        mb.append(None)
        wf[("Lmax", ky)] = ml
    wmax = pool.tile([128, 1], f32, name="wmax")
    nc.vector.tensor_tensor(wmax[:], mb[0][:], mb[2][:], op=mybir.AluOpType.max)
    nc.vector.tensor_tensor(wmax[:], wmax[:], mb[4][:], op=mybir.AluOpType.max)
    for ky in range(3):
        nc.vector.tensor_tensor(wmax[0:64, :], wmax[0:64, :],
                                wf[("Lmax", ky)][0:64, :], op=mybir.AluOpType.max)
    wbound = pool.tile([128, 1], f32, name="wbound")
    nc.gpsimd.partition_all_reduce(wbound[:], wmax[:], channels=128,
                                   reduce_op=bass_isa.ReduceOp.max)
    return wf, wbound


def _quantize_tile(nc, tmp_pool, dst, src, scale_ap, engine="vector",
                   magic_ap=None):
    """dst(bf16) = round_half_even(src * scale), via the fp32 magic constant."""
    p = src.shape[0]
    tmpq = tmp_pool.tile(list(src.shape), f32, name="tmpq")
    if engine == "scalar":
        nc.scalar.activation(tmpq[:], src, mybir.ActivationFunctionType.Identity,
                             bias=magic_ap[0:p, :], scale=scale_ap[0:p, :])
    else:
        nc.vector.tensor_scalar(tmpq[:], src, scale_ap[0:p, :], MAGIC,
                                op0=mybir.AluOpType.mult, op1=mybir.AluOpType.add)
    nc.vector.tensor_scalar(dst, tmpq[:], MAGIC, None,
                            op0=mybir.AluOpType.subtract)


def build_bass(n_cores=N_CORES, n_img=N_PER_CORE, repeat=1):
    nc = bacc.Bacc("TRN2", target_bir_lowering=False, debug=False,
                   num_devices=n_cores)
    lhs = nc.dram_tensor("lhs", [n_img, H, W, C], f32, kind="ExternalInput").ap()
    rhs = nc.dram_tensor("rhs", [3, 3, C, CO], f32, kind="ExternalInput").ap()
    wts = nc.dram_tensor("weights", [n_img, 1, 1, 1], f32, kind="ExternalInput").ap()
    out = nc.dram_tensor("out", [n_img, H, W, CO], f32, kind="ExternalOutput").ap()

    with tile.TileContext(nc) as tc:
        for _ in range(repeat):
            _body(tc, nc, lhs, rhs, wts, out, n_cores, n_img)
    nc.compile()
    return nc


def _body(tc, nc, lhs, rhs, wts, out, n_cores, n_img):
    from contextlib import ExitStack
    with ExitStack() as ctx:
        cpool = ctx.enter_context(tc.tile_pool(name="cpool", bufs=1))
        wq_pool = ctx.enter_context(tc.tile_pool(name="wq", bufs=1))
        trow_pool = ctx.enter_context(tc.tile_pool(name="trow", bufs=2))
        tmp_pool = ctx.enter_context(tc.tile_pool(name="tmpq", bufs=2))
        tq_pool = ctx.enter_context(tc.tile_pool(name="tq", bufs=2))
        tc_pool = ctx.enter_context(tc.tile_pool(name="tcq", bufs=2))
        st_pool = ctx.enter_context(tc.tile_pool(name="stg", bufs=3))
        sti_pool = ctx.enter_context(tc.tile_pool(name="sti", bufs=4))
        ps_pool = ctx.enter_context(tc.tile_pool(name="ps", bufs=6, space="PSUM"))
        ps2_pool = ctx.enter_context(tc.tile_pool(name="ps2", bufs=2, space="PSUM"))
        dram_pool = ctx.enter_context(tc.tile_pool(name="dram", bufs=1, space="DRAM"))

        # ---- constants
        c127 = cpool.tile([128, 1], f32, name="c127")
        nc.vector.memset(c127[:], CLIPV)
        c1 = cpool.tile([128, 1], f32, name="c1")
        nc.vector.memset(c1[:], 1.0)
        cmagic = cpool.tile([128, 1], f32, name="cmagic")
        nc.vector.memset(cmagic[:], MAGIC)

        # ---- filter prep (local; every core computes the same scale)
        wf, wbound = _weight_prep(nc, cpool, wq_pool, rhs)

        # ---- streamed masked abs-max over lhs (chunks; rows not kept)
        mcat = cpool.tile([1, max(n_img, 2)], f32, name="mcat")
        lhs_rows = lhs.rearrange("n y x c -> n y (x c)")  # [n, 112, 7168]
        for i in range(n_img):
            ms = []
            for h in range(2):
                t = trow_pool.tile([H, HALF], f32, tag="trow", name="trowa")
                nc.sync.dma_start(t[:], lhs_rows[i][:, h * HALF:(h + 1) * HALF])
                m = cpool.tile([H, 1], f32, name=f"lm{i}_{h}")
                nc.vector.tensor_reduce(m[:], t[:], axis=mybir.AxisListType.X,
                                        op=mybir.AluOpType.max,
                                        apply_absolute_value=True)
                ms.append(m)
            nc.vector.tensor_tensor(ms[0][:], ms[0][:], ms[1][:],
                                    op=mybir.AluOpType.max)
            pm = cpool.tile([H, 1], f32, name=f"lpm{i}")
            nc.gpsimd.partition_all_reduce(pm[:], ms[0][:], channels=H,
                                           reduce_op=bass_isa.ReduceOp.max)
            nc.vector.tensor_copy(mcat[0:1, i:i + 1], pm[0:1, :])

        # mask = weights > 0 (per image)
        wcal = cpool.tile([1, max(n_img, 2)], f32, name="wcal")
        nc.sync.dma_start(wcal[0:1, 0:n_img],
                          wts.rearrange("n a b c -> (a b c) n"))
        ind = cpool.tile([1, max(n_img, 2)], f32, name="ind")
        nc.vector.tensor_scalar(ind[0:1, 0:n_img], wcal[0:1, 0:n_img], 0.0, None,
                                op0=mybir.AluOpType.is_gt)
        masked = cpool.tile([1, max(n_img, 2)], f32, name="masked")
        nc.vector.tensor_tensor(masked[0:1, 0:n_img], mcat[0:1, 0:n_img],
                                ind[0:1, 0:n_img], op=mybir.AluOpType.mult)
        lb = cpool.tile([1, 1], f32, name="lb")
        nc.vector.tensor_reduce(lb[:], masked[0:1, 0:n_img],
                                axis=mybir.AxisListType.X, op=mybir.AluOpType.max)

        # ---- global max across cores
        if n_cores > 1 and 'nocc' not in ABL:
            cc_in = dram_pool.tile([1, 1], f32, name="cc_in")
            cc_space = "Shared" if n_cores > 4 else "Local"
            cc_out = dram_pool.tile([1, 1], f32, name="cc_out",
                                    addr_space=cc_space)
            nc.gpsimd.dma_start(cc_in[:], lb[:])
            nc.gpsimd.collective_compute(
                "AllReduce", mybir.AluOpType.max,
                replica_groups=[list(range(n_cores))],
                ins=[cc_in.opt()], outs=[cc_out.opt()])
            gb0 = cpool.tile([1, 1], f32, name="gb0")
            nc.gpsimd.dma_start(gb0[:], cc_out[:])
        else:
            gb0 = lb

        gbb = cpool.tile([128, 1], f32, name="gbb")
        nc.gpsimd.partition_broadcast(gbb[:], gb0[:])
        gb2 = cpool.tile([128, 1], f32, name="gb2")
        nc.vector.tensor_scalar(gb2[:], gbb[:], EPS, None,
                                op0=mybir.AluOpType.max)
        rls = cpool.tile([128, 1], f32, name="rls")
        nc.vector.reciprocal(rls[:], gb2[:])
        ls = cpool.tile([128, 1], f32, name="ls")  # lhs_scale = 127/bound
        nc.vector.tensor_scalar(ls[:], rls[:], CLIPV, None,
                                op0=mybir.AluOpType.mult)

        wb2 = cpool.tile([128, 1], f32, name="wb2")
        nc.vector.tensor_scalar(wb2[:], wbound[:], EPS, None,
                                op0=mybir.AluOpType.max)
        rws = cpool.tile([128, 1], f32, name="rws")
        nc.vector.reciprocal(rws[:], wb2[:])
        ws = cpool.tile([128, 1], f32, name="ws")   # rhs_scale
        nc.vector.tensor_scalar(ws[:], rws[:], CLIPV, None,
                                op0=mybir.AluOpType.mult)
        sprod = cpool.tile([128, 1], f32, name="sprod")
        nc.vector.tensor_tensor(sprod[:], ls[:], ws[:], op=mybir.AluOpType.mult)
        dq = cpool.tile([128, 1], f32, name="dq")   # 1/(ls*ws)
        nc.vector.reciprocal(dq[:], sprod[:])

        # ---- quantize filter chunks -> bf16 lhsT tiles
        wq = {}
        for key_kind in ("A", "B", "U", "L"):
            for ky in range(3):
                src = wf[(key_kind, ky)]
                dst = wq_pool.tile([128, 128], bf16, name=f"wq{key_kind}{ky}")
                if key_kind == "U":
                    _quantize_tile(nc, tmp_pool, dst[64:128, :],
                                   src[64:128, :], ws[64:128, :])
                elif key_kind == "L":
                    _quantize_tile(nc, tmp_pool, dst[0:64, :],
                                   src[0:64, :], ws)
                else:
                    _quantize_tile(nc, tmp_pool, dst[:], src[:], ws)
                wq[(key_kind, ky)] = dst

        wzero = wq_pool.tile([128, 128], bf16, name="wzero")
        nc.vector.memset(wzero[:], 0.0)

        # output view: x = 2*m + parity; dims (n, parity, c, y, m)
        from concourse.masks import make_identity
        ident = cpool.tile([128, 128], f32, name="ident")
        make_identity(nc, ident[:])

        # ---- per image: reload lhs -> quantize -> xbar-transpose into TC -> conv
        for i in range(n_img):
            tcq = tc_pool.tile([128, TCW], bf16, name="tcq")
            # j-major views [p, j', y]; transpose dst contiguous per partition
            tcr = tcq.rearrange("p (j r) -> p j r", r=RSLOTS)  # [128,58,112]
            if 'noms' not in ABL:
                nc.gpsimd.memset(tcr[:, 0, :], 0.0)   # x=-1 pad column
                nc.gpsimd.memset(tcr[:, 57, :], 0.0)  # x=112 pad column
            for h in range(2):
                trow = trow_pool.tile([H, HALF], f32, tag="trow", name="trowb")
                nc.sync.dma_start(trow[:],
                                  lhs_rows[i][:, h * HALF:(h + 1) * HALF])
                tq = tq_pool.tile([H, HALF], bf16, name="tq")
                if 'noq' not in ABL:
                    _quantize_tile(nc, tmp_pool, tq[:], trow[:], ls,
                                   engine="vector", magic_ap=cmagic)
                if 'notr' not in ABL:
                    nc.sync.dma_start_transpose(
                        tcr[:, 1 + 28 * h: 1 + 28 * (h + 1), :], tq[:])

            tcr2 = tcq.rearrange("p (j r) -> p r j", r=RSLOTS)  # [128,112,58]

            def clip(g, ky):
                lo_in = g * G_ROWS + ky - 1
                lo, hi = max(0, lo_in), min(H, lo_in + G_ROWS)
                return lo, hi, lo - lo_in, hi - lo_in

            KYS = (1, 0, 2)  # ky=1 first: full rows, starts the accumulation
            for sg0 in range(0, N_G, SG):
                gs = range(sg0, min(sg0 + SG, N_G))
                pse = {}
                pso = {}
                for g in gs:
                    pse[g] = ps_pool.tile([128, G_ROWS, 56], f32, name="pse",
                                          tag="psb")
                    pso[g] = ps_pool.tile([128, G_ROWS, 56], f32, name="pso",
                                          tag="psb")
                # full K=128 chunks (A: even-parity outputs, B: odd)
                for kind, psd in (() if 'nomm' in ABL else (("A", pse), ("B", pso))):
                    for ky in KYS:
                        for g in gs:
                            lo, hi, a, b = clip(g, ky)
                            nc.tensor.matmul(
                                psd[g][:, a:b, :], wq[(kind, ky)][:],
                                tcr2[:, lo:hi, 1:57],
                                start=(ky == 1), stop=False,
                                skip_group_check=True)
                # half chunks: U (upper rows, even) / L (lower rows, odd);
                # adjacent issue -> disjoint PE row-groups run concurrently
                for ky in (() if 'nomm' in ABL else KYS):
                    for g in gs:
                        lo, hi, a, b = clip(g, ky)
                        nc.tensor.matmul(
                            pse[g][:, a:b, :], wq[("U", ky)][64:128, :],
                            tcr2[64:128, lo:hi, 0:56],
                            start=False, stop=(ky == 2),
                            skip_group_check=True)
                        nc.tensor.matmul(
                            pso[g][:, a:b, :], wq[("L", ky)][0:64, :],
                            tcr2[0:64, lo:hi, 2:58],
                            start=False, stop=(ky == 2),
                            skip_group_check=True)
                # dequant, TensorE-transpose to [spatial, ch], store
                for g in (() if 'noout' in ABL else gs):
                    hh, gl = divmod(g, 7)
                    if gl == 0:
                        stimg = [sti_pool.tile([112, 7, 4, 128], f32, tag="sti",
                                               name=f"sti{p}") for p in (0, 1)]
                    for parity, ps in ((0, pse[g]), (1, pso[g])):
                        st = st_pool.tile([128, 448], f32, name="stg")
                        if 'noact' not in ABL:
                            nc.scalar.activation(st[:], ps.rearrange("p a b -> p (a b)"),
                                                 mybir.ActivationFunctionType.Copy,
                                                 scale=dq[:])
                        ps2 = ps2_pool.tile([112, 512], f32, name="ps2")
                        if 'notp' not in ABL:
                            for t in range(4):
                                nc.tensor.transpose(ps2[:, 128 * t:128 * (t + 1)],
                                                    st[:, 112 * t:112 * (t + 1)],
                                                    ident[:])
                            dstf = stimg[parity][:, gl].rearrange("p a b -> p (a b)")
                            if parity == 0:
                                nc.vector.tensor_copy(dstf, ps2[:])
                            else:
                                nc.scalar.copy(dstf, ps2[:])
                    # one DMA per finished half-image per parity
                    # out[i, y, x, c]: y = 56*hh + 8*gl + 2*t + dy, x = 2*m+parity
                    if gl == 6 and 'nodma' not in ABL:
                        for parity in (0, 1):
                            dview = out[i, hh * 56:(hh + 1) * 56] \
                                .rearrange("(g t dy) (m two) c -> two (dy m) g t c",
                                           g=7, t=4, two=2)[parity]
                            nc.sync.dma_start(dview, stimg[parity][:])


_NC_CACHE = {}


def _get_nc(repeat=1):
    key = (N_CORES, N_PER_CORE, repeat)
    if key not in _NC_CACHE:
        _NC_CACHE[key] = build_bass(N_CORES, N_PER_CORE, repeat=repeat)
    return _NC_CACHE[key]


def kernel(lhs, rhs, weights, event_count=None, event_count_for_filter=None,
           **_ignored):
    lhs = np.ascontiguousarray(np.asarray(lhs, dtype=np.float32))
    rhs = np.ascontiguousarray(np.asarray(rhs, dtype=np.float32))
    weights = np.ascontiguousarray(np.asarray(weights, dtype=np.float32))
    n = lhs.shape[0]
    per = n // N_CORES
    assert per == N_PER_CORE and n % N_CORES == 0

    nc = _get_nc()
    in_maps = []
    for c in range(N_CORES):
        in_maps.append({
            "lhs": lhs[c * per:(c + 1) * per],
            "rhs": rhs,
            "weights": weights[c * per:(c + 1) * per],
        })
    res = bass_utils.run_bass_kernel_spmd(nc, in_maps,
                                          core_ids=list(range(N_CORES)))
    return np.concatenate([res.results[c]["out"] for c in range(N_CORES)],
                          axis=0)

